# revision 11
# baseline (speedup 1.0000x reference)
"""Trainium2 Bass kernel for the GNN message-passing problem.

Math (from the reference, already algebraically collapsed):
    h        = x @ W_node                                  [B, N, O]
    new_x    = N*h@W_i + (sum_n h[n])@W_j + (sum_j adj[:,:,j,:])@W_e
               + N*b_edge + h
    output   = (new_x, adj)          # adj passes through untouched

Shapes: B=4, N=512, F=256, E=8, O=128.  adj is 33.5 MB — the dominant
stream; everything else is small.  target_regime = memory.

Sharding: 8 cores = (batch b = c//2) x (i-half = c%2).  Each core:
  - streams its adj shard [256, 512, 8] (4 MB) and j-reduces it on DVE
  - computes h for the whole batch (needed for the sum_j h term)
  - emits out rows [256, 128]

SPMD trick: per-core x is rolled so that rows 0:256 are always the
core's own rows — the program is identical across cores, only data
differs.  sum_n h is rotation-invariant.
"""

import numpy as np

import concourse.bass as bass
import concourse.tile as tile
from concourse import bacc
from concourse import mybir
from concourse.bass_utils import run_bass_kernel_spmd

F32 = mybir.dt.float32

B, N, F_NODE, F_EDGE, F_OUT = 4, 512, 256, 8, 128
IH = N // 2          # rows per core = 256
JC = 256             # adj j-chunk size (per-DMA tile is [128, JC, 8] = 1 MB)
NJC = N // JC        # j-chunks


def build_bass():
    nc = bacc.Bacc("TRN2", target_bir_lowering=False)

    adj_d = nc.declare_dram_parameter("adj_s", [IH, N, F_EDGE], F32, isOutput=False)
    x_d = nc.declare_dram_parameter("x_r", [N, F_NODE], F32, isOutput=False)
    wn_d = nc.declare_dram_parameter("wn", [F_NODE, F_OUT], F32, isOutput=False)
    wi_d = nc.declare_dram_parameter("wi", [F_OUT, F_OUT], F32, isOutput=False)
    wj_d = nc.declare_dram_parameter("wj", [F_OUT, F_OUT], F32, isOutput=False)
    we_d = nc.declare_dram_parameter("we", [F_EDGE, F_OUT], F32, isOutput=False)
    be_d = nc.declare_dram_parameter("be", [1, F_OUT], F32, isOutput=False)
    id_d = nc.declare_dram_parameter("ident", [128, 128], F32, isOutput=False)
    out_d = nc.declare_dram_parameter("out", [IH, F_OUT], F32, isOutput=True)

    with tile.TileContext(nc) as tc:
        with (
            tc.tile_pool(name="const", bufs=1) as const,
            tc.tile_pool(name="adj", bufs=4) as adj_pool,
            tc.tile_pool(name="work", bufs=2) as work,
            tc.tile_pool(name="sred", bufs=4) as sred,
            tc.tile_pool(name="stt", bufs=4) as stt,
            tc.tile_pool(name="singles", bufs=1) as singles,
            tc.tile_pool(name="ps_xt", bufs=1, space="PSUM") as ps_xt,
            tc.tile_pool(name="ps_ht", bufs=1, space="PSUM") as ps_ht,
            tc.tile_pool(name="ps_st", bufs=3, space="PSUM") as ps_st,
            tc.tile_pool(name="ps_mj", bufs=1, space="PSUM") as ps_mj,
            tc.tile_pool(name="ps_res", bufs=2, space="PSUM") as ps_res,
        ):
            # ---- constants ----
            ident = const.tile([128, 128], F32)
            nc.sync.dma_start(out=ident, in_=id_d[:])
            ones_row = const.tile([1, 128], F32)
            nc.vector.memset(ones_row, 1.0)

            # ---- weight loads ----
            wn_sb = const.tile([128, 2, F_OUT], F32)   # [f%128, f//128, o]
            nc.sync.dma_start(
                out=wn_sb, in_=wn_d[:].rearrange("(c p) o -> p c o", p=128)
            )
            wi_sb = const.tile([128, 128], F32)
            nc.sync.dma_start(out=wi_sb, in_=wi_d[:])
            wj_sb = const.tile([128, 128], F32)
            nc.sync.dma_start(out=wj_sb, in_=wj_d[:])
            we_sb = const.tile([F_EDGE, 128], F32)
            nc.sync.dma_start(out=we_sb, in_=we_d[:])
            be_sb = const.tile([1, 128], F32)
            nc.sync.dma_start(out=be_sb, in_=be_d[:])

            # ---- PE warmup chain: bring each const's DMA sem into the
            # PE vector clock, one new semaphore per instruction (the
            # fp32 transpose-mode LDWEIGHTS carries only one sync-wait).
            warm = ps_xt.tile([128, N], F32, tag="xt_ps")
            nc.tensor.matmul(warm[:, 0:128], lhsT=ident, rhs=ident, start=True, stop=True)
            nc.tensor.matmul(warm[:, 128:256], lhsT=wi_sb, rhs=ident, start=True, stop=True)
            nc.tensor.matmul(warm[:, 256:384], lhsT=wj_sb, rhs=ident, start=True, stop=True)
            nc.tensor.matmul(warm[:, 384:512], lhsT=wn_sb[:, 0, :], rhs=ident, start=True, stop=True)
            nc.tensor.matmul(
                warm[:, 0:128], lhsT=we_sb, rhs=we_sb, start=True, stop=True
            )

            # ---- adj stream + j-reduction (the bulk of the kernel) ----
            # ST_sb[ih][jc] holds S_chunk^T = (sum_{j in chunk} adj[i, j, :]).T
            # laid out [e=8 partitions, i=128 free] ready to be matmul lhsT.
            st_tiles = []
            for ih in range(2):
                for jc in range(NJC):
                    adj_t = adj_pool.tile([128, JC, F_EDGE], F32, tag="adjt")
                    nc.sync.dma_start(
                        out=adj_t,
                        in_=adj_d[ih * 128 : (ih + 1) * 128, jc * JC : (jc + 1) * JC, :],
                    )
                    s_sb = sred.tile([128, F_EDGE], F32, tag="s_sb")
                    nc.vector.reduce_sum(
                        out=s_sb,
                        in_=adj_t.rearrange("p j e -> p e j"),
                        axis=mybir.AxisListType.X,
                    )
                    st_ps = ps_st.tile([F_EDGE, 128], F32, tag="st_ps")
                    nc.tensor.matmul(st_ps, lhsT=s_sb, rhs=ident[:, :], start=True, stop=True)
                    st_sb = stt.tile([F_EDGE, 128], F32, tag="st_sb")
                    nc.scalar.copy(st_sb, st_ps)
                    st_tiles.append((ih, st_sb))

            # ---- x load + transpose (x^T needed: contraction is over f) ----
            x_sb = work.tile([128, 4, F_NODE], F32, tag="x_sb")
            nc.sync.dma_start(
                out=x_sb, in_=x_d[:].rearrange("(c p) f -> p c f", p=128)
            )
            xT_sb = []
            for c in range(2):  # f-half
                xt_ps = ps_xt.tile([128, N], F32, tag="xt_ps")
                for nc_i in range(4):  # n-chunk
                    nc.tensor.matmul(
                        xt_ps[:, nc_i * 128 : (nc_i + 1) * 128],
                        lhsT=x_sb[:, nc_i, c * 128 : (c + 1) * 128],
                        rhs=ident,
                        start=True, stop=True,
                    )
                xt = work.tile([128, N], F32, tag=f"xT_{c}")
                nc.scalar.copy(xt, xt_ps)
                xT_sb.append(xt)

            # ---- h^T = (x @ W_node)^T : [o=128, n=512] ----
            hT_ps = ps_ht.tile([128, N], F32, tag="hT_ps")
            for c in range(2):
                nc.tensor.matmul(
                    hT_ps, lhsT=wn_sb[:, c, :], rhs=xT_sb[c],
                    start=(c == 0), stop=(c == 1),
                )
            hT_sb = work.tile([128, N], F32, tag="hT_sb")
            nc.scalar.copy(hT_sb, hT_ps)

            # ---- bias pieces: msg_j = (sum_n h[n]) @ W_j, and N*b_edge ----
            hsum_col = singles.tile([128, 1], F32, tag="hsum")
            nc.vector.reduce_sum(hsum_col, hT_sb, axis=mybir.AxisListType.X)
            msgj_ps = ps_mj.tile([1, 128], F32, tag="msgj")
            nc.tensor.matmul(msgj_ps, lhsT=hsum_col, rhs=wj_sb, start=True, stop=True)
            msgj_sb = singles.tile([1, 128], F32, tag="msgj_sb")
            nc.scalar.copy(msgj_sb, msgj_ps)
            be_n = singles.tile([1, 128], F32, tag="be_n")
            nc.scalar.mul(be_n, be_sb, float(N))

            wi_s = singles.tile([128, 128], F32, tag="wi_s")
            nc.scalar.mul(wi_s, wi_sb, float(N))

            # ---- result per i-chunk, all in one PSUM accumulation group:
            #   h@(N*W_i) + h@I + sum_c S_c@W_e + 1x(N*b_edge) + 1x(msg_j)
            for ih in range(2):
                hT_sl = hT_sb[:, ih * 128 : (ih + 1) * 128]
                res_ps = ps_res.tile([128, 128], F32, tag="res")
                nc.tensor.matmul(res_ps, lhsT=hT_sl, rhs=wi_s, start=True, stop=False)
                nc.tensor.matmul(res_ps, lhsT=hT_sl, rhs=ident, start=False, stop=False)
                for ih_s, st_sb in st_tiles:
                    if ih_s == ih:
                        nc.tensor.matmul(
                            res_ps, lhsT=st_sb, rhs=we_sb, start=False, stop=False
                        )
                nc.tensor.matmul(
                    res_ps, lhsT=ones_row, rhs=be_n, start=False, stop=False
                )
                nc.tensor.matmul(
                    res_ps, lhsT=ones_row, rhs=msgj_sb, start=False, stop=True
                )
                out_sb = work.tile([128, 128], F32, tag="out_sb")
                nc.scalar.copy(out_sb, res_ps)
                nc.sync.dma_start(
                    out=out_d[ih * 128 : (ih + 1) * 128, :], in_=out_sb
                )

    nc.compile()
    return nc


_NC = None


def _get_nc():
    global _NC
    if _NC is None:
        _NC = build_bass()
    return _NC


def make_in_maps(x, adj, W_node, W_edge, b_edge):
    x = np.asarray(x, np.float32)
    adj = np.asarray(adj, np.float32)
    W_node = np.ascontiguousarray(np.asarray(W_node, np.float32))
    W_edge = np.asarray(W_edge, np.float32)
    b_edge = np.asarray(b_edge, np.float32)
    in_maps = []
    for c in range(8):
        b, ihalf = c // 2, c % 2
        i0 = ihalf * IH
        in_maps.append(
            {
                "adj_s": np.ascontiguousarray(adj[b, i0 : i0 + IH]),
                "x_r": np.ascontiguousarray(np.roll(x[b], -i0, axis=0)),
                "wn": W_node,
                "wi": np.ascontiguousarray(W_edge[0:F_OUT]),
                "wj": np.ascontiguousarray(W_edge[F_OUT : 2 * F_OUT]),
                "we": np.ascontiguousarray(W_edge[2 * F_OUT :]),
                "be": np.ascontiguousarray(b_edge.reshape(1, F_OUT)),
                "ident": np.eye(128, dtype=np.float32),
            }
        )
    return in_maps


def run(x, adj, W_node, W_edge, b_edge, **run_kwargs):
    """Run on 8 neuron cores; returns (new_x, adj, BassKernelResults)."""
    nc = _get_nc()
    in_maps = make_in_maps(x, adj, W_node, W_edge, b_edge)
    res = run_bass_kernel_spmd(nc, in_maps, list(range(8)), **run_kwargs)
    new_x = np.empty((B, N, F_OUT), np.float32)
    for c in range(8):
        b, ihalf = c // 2, c % 2
        new_x[b, ihalf * IH : (ihalf + 1) * IH] = res.results[c]["out"]
    return new_x, res


def kernel(x, adj, W_node, W_edge, b_edge):
    new_x, _ = run(x, adj, W_node, W_edge, b_edge)
    return new_x, np.asarray(adj)


# revision 13
# speedup vs baseline: 1.2437x; 1.2437x over previous
"""Trainium2 Bass kernel for the GNN message-passing problem.

Math (from the reference, already algebraically collapsed):
    h        = x @ W_node                                  [B, N, O]
    new_x    = N*h@W_i + (sum_n h[n])@W_j + (sum_j adj[:,:,j,:])@W_e
               + N*b_edge + h
    output   = (new_x, adj)          # adj passes through untouched

Shapes: B=4, N=512, F=256, E=8, O=128.  adj is 33.5 MB — the dominant
stream; everything else is small.  target_regime = memory.

Sharding: 8 cores = (batch b = c//2) x (i-half = c%2).  Each core:
  - streams its adj shard [256, 512, 8] (4 MB); the j-reduction is
    split between GPSIMD (contiguous level-1 fold) and DVE (strided
    tensor_reduce of the folded tile) so both engines stay under the
    DMA stream time
  - computes h for the whole batch (needed for the sum_j h term)
  - emits out rows [256, 128]

Precision: the 512-term j-reduction (the numerically heavy part) is
fp32 end-to-end; only the small matmuls run in bf16 (fp32 matmuls on
TRN2 lower to a 2-pass LOW/HIGH mode that is ~10x slower).

SPMD trick: per-core x is rolled so that rows 0:256 are always the
core's own rows, and pre-transposed on the host (pure layout
marshaling) since the contraction dim f must sit on partitions.
"""

import numpy as np
import ml_dtypes

import concourse.bass as bass
import concourse.tile as tile
from concourse import bacc, mybir
from concourse.bass_utils import run_bass_kernel_spmd

F32 = mybir.dt.float32
BF16 = mybir.dt.bfloat16

B, N, F_NODE, F_EDGE, F_OUT = 4, 512, 256, 8, 128
IH = N // 2          # rows per core = 256
JC = 128             # adj j-chunk (tile [128, JC, 8] = 512 KB per DMA)
NJC = N // JC        # j-chunks per i-half


def build_bass():
    nc = bacc.Bacc("TRN2", target_bir_lowering=False)

    adj_d = nc.declare_dram_parameter("adj_s", [IH, N, F_EDGE], F32, isOutput=False)
    xt_d = nc.declare_dram_parameter("xT_r", [F_NODE, N], F32, isOutput=False)
    wn_d = nc.declare_dram_parameter("wn", [F_NODE, F_OUT], F32, isOutput=False)
    wi_d = nc.declare_dram_parameter("wi", [F_OUT, F_OUT], F32, isOutput=False)
    wj_d = nc.declare_dram_parameter("wj", [F_OUT, F_OUT], F32, isOutput=False)
    we_d = nc.declare_dram_parameter("we", [F_EDGE, F_OUT], F32, isOutput=False)
    be_d = nc.declare_dram_parameter("be", [1, F_OUT], F32, isOutput=False)
    id_d = nc.declare_dram_parameter("identb", [128, 128], BF16, isOutput=False)
    out_d = nc.declare_dram_parameter("out", [IH, F_OUT], F32, isOutput=True)

    with tile.TileContext(nc) as tc:
        with (
            tc.tile_pool(name="const", bufs=1) as const,
            tc.tile_pool(name="adj", bufs=4) as adj_pool,
            tc.tile_pool(name="fold", bufs=4) as fold_pool,
            tc.tile_pool(name="work", bufs=2) as work,
            tc.tile_pool(name="sred", bufs=4) as sred,
            tc.tile_pool(name="stt", bufs=8) as stt,
            tc.tile_pool(name="singles", bufs=1) as singles,
            tc.tile_pool(name="ps_ht", bufs=1, space="PSUM") as ps_ht,
            tc.tile_pool(name="ps_st", bufs=4, space="PSUM") as ps_st,
            tc.tile_pool(name="ps_mj", bufs=1, space="PSUM") as ps_mj,
            tc.tile_pool(name="ps_res", bufs=2, space="PSUM") as ps_res,
        ):
            # ---- small input DMAs first so they land before the stream ----
            identb = const.tile([128, 128], BF16)
            nc.sync.dma_start(out=identb, in_=id_d[:])
            wn_f = const.tile([128, 2, F_OUT], F32)     # [f%128, f//128, o]
            nc.sync.dma_start(
                out=wn_f, in_=wn_d[:].rearrange("(c p) o -> p c o", p=128)
            )
            wi_f = const.tile([128, 128], F32)
            nc.sync.dma_start(out=wi_f, in_=wi_d[:])
            wj_f = const.tile([128, 128], F32)
            nc.sync.dma_start(out=wj_f, in_=wj_d[:])
            we_f = const.tile([F_EDGE, 128], F32)
            nc.sync.dma_start(out=we_f, in_=we_d[:])
            be_f = const.tile([1, 128], F32)
            nc.sync.dma_start(out=be_f, in_=be_d[:])
            xt_f = work.tile([128, 2, N], F32, tag="xt_f")  # [f%128, f//128, n]
            nc.sync.dma_start(
                out=xt_f, in_=xt_d[:].rearrange("(c p) n -> p c n", p=128)
            )
            ones_row = const.tile([1, 128], BF16)
            nc.vector.memset(ones_row, 1.0)

            # ---- device casts f32 -> bf16 (ScalarE is otherwise idle) ----
            wn_b = const.tile([128, 2, F_OUT], BF16)
            nc.scalar.copy(wn_b, wn_f)
            wi_s = const.tile([128, 128], BF16)
            nc.scalar.mul(wi_s, wi_f, float(N))        # N*W_i (2^9: exact)
            wj_b = const.tile([128, 128], BF16)
            nc.scalar.copy(wj_b, wj_f)
            we_b = const.tile([F_EDGE, 128], BF16)
            nc.scalar.copy(we_b, we_f)
            be_n = const.tile([1, 128], BF16)
            nc.scalar.mul(be_n, be_f, float(N))        # N*b_edge
            xt_b = work.tile([128, 2, N], BF16, tag="xt_b")
            nc.scalar.copy(xt_b, xt_f)

            # ---- h^T = (x @ W_node)^T : [o=128, n=512], bf16 matmuls ----
            hT_ps = ps_ht.tile([128, N], F32, tag="hT_ps")
            for c in range(2):
                nc.tensor.matmul(
                    hT_ps, lhsT=wn_b[:, c, :], rhs=xt_b[:, c, :],
                    start=(c == 0), stop=(c == 1),
                )
            hT_b = work.tile([128, N], BF16, tag="hT_b")
            nc.scalar.copy(hT_b, hT_ps)

            # ---- msg_j = (sum_n h[n]) @ W_j  (one row) ----
            hsum_f = singles.tile([128, 1], F32, tag="hsum")
            nc.vector.reduce_sum(hsum_f, hT_b, axis=mybir.AxisListType.X)
            hsum_b = singles.tile([128, 1], BF16, tag="hsum_b")
            nc.scalar.copy(hsum_b, hsum_f)
            msgj_ps = ps_mj.tile([1, 128], F32, tag="msgj")
            nc.tensor.matmul(msgj_ps, lhsT=hsum_b, rhs=wj_b, start=True, stop=True)
            msgj_b = singles.tile([1, 128], BF16, tag="msgj_b")
            nc.scalar.copy(msgj_b, msgj_ps)

            # ---- adj stream + j-reduction (the bulk of the kernel) ----
            # Per chunk: GPSIMD folds j in half (contiguous adds), DVE
            # strided-reduces the rest, PE transposes S_chunk to put e on
            # partitions.  All fp32 until after the reduction.
            st_tiles = []
            for ih in range(2):
                for jc in range(NJC):
                    adj_t = adj_pool.tile([128, JC, F_EDGE], F32, tag="adjt")
                    nc.sync.dma_start(
                        out=adj_t,
                        in_=adj_d[ih * 128 : (ih + 1) * 128, jc * JC : (jc + 1) * JC, :],
                    )
                    half = JC * F_EDGE // 2     # 512 elems
                    flat = adj_t.rearrange("p j e -> p (j e)")
                    fold_t = fold_pool.tile([128, JC // 2, F_EDGE], F32, tag="fold")
                    nc.gpsimd.tensor_tensor(
                        fold_t.rearrange("p j e -> p (j e)"),
                        flat[:, 0:half], flat[:, half : 2 * half],
                        mybir.AluOpType.add,
                    )
                    s_f = sred.tile([128, F_EDGE], F32, tag="s_f")
                    nc.vector.reduce_sum(
                        out=s_f,
                        in_=fold_t.rearrange("p j e -> p e j"),
                        axis=mybir.AxisListType.X,
                    )
                    s_b = sred.tile([128, F_EDGE], BF16, tag="s_b")
                    nc.scalar.copy(s_b, s_f)
                    st_ps = ps_st.tile([F_EDGE, 128], BF16, tag="st_ps")
                    nc.tensor.transpose(st_ps, s_b, identb)
                    st_b = stt.tile([F_EDGE, 128], BF16, tag="st_b")
                    nc.scalar.copy(st_b, st_ps)
                    st_tiles.append((ih, st_b))

            # ---- result per i-chunk, one PSUM accumulation group:
            #   h@(N*W_i) + h@I + sum_c S_c@W_e + 1x(N*b_edge) + 1x(msg_j)
            for ih in range(2):
                hT_sl = hT_b[:, ih * 128 : (ih + 1) * 128]
                res_ps = ps_res.tile([128, 128], F32, tag="res")
                nc.tensor.matmul(res_ps, lhsT=hT_sl, rhs=wi_s, start=True, stop=False)
                nc.tensor.matmul(res_ps, lhsT=hT_sl, rhs=identb, start=False, stop=False)
                for ih_s, st_b in st_tiles:
                    if ih_s == ih:
                        nc.tensor.matmul(
                            res_ps, lhsT=st_b, rhs=we_b, start=False, stop=False
                        )
                nc.tensor.matmul(
                    res_ps, lhsT=ones_row, rhs=be_n, start=False, stop=False
                )
                nc.tensor.matmul(
                    res_ps, lhsT=ones_row, rhs=msgj_b, start=False, stop=True
                )
                out_sb = work.tile([128, 128], F32, tag="out_sb")
                nc.scalar.copy(out_sb, res_ps)
                nc.sync.dma_start(
                    out=out_d[ih * 128 : (ih + 1) * 128, :], in_=out_sb
                )

    nc.compile()
    return nc


_NC = None


def _get_nc():
    global _NC
    if _NC is None:
        _NC = build_bass()
    return _NC


def make_in_maps(x, adj, W_node, W_edge, b_edge):
    x = np.asarray(x, np.float32)
    adj = np.asarray(adj, np.float32)
    W_node = np.ascontiguousarray(np.asarray(W_node, np.float32))
    W_edge = np.asarray(W_edge, np.float32)
    b_edge = np.asarray(b_edge, np.float32)
    identb = np.eye(128, dtype=ml_dtypes.bfloat16)
    in_maps = []
    for c in range(8):
        b, ihalf = c // 2, c % 2
        i0 = ihalf * IH
        in_maps.append(
            {
                "adj_s": np.ascontiguousarray(adj[b, i0 : i0 + IH]),
                "xT_r": np.ascontiguousarray(np.roll(x[b], -i0, axis=0).T),
                "wn": W_node,
                "wi": np.ascontiguousarray(W_edge[0:F_OUT]),
                "wj": np.ascontiguousarray(W_edge[F_OUT : 2 * F_OUT]),
                "we": np.ascontiguousarray(W_edge[2 * F_OUT :]),
                "be": np.ascontiguousarray(b_edge.reshape(1, F_OUT)),
                "identb": identb,
            }
        )
    return in_maps


def run(x, adj, W_node, W_edge, b_edge, **run_kwargs):
    """Run on 8 neuron cores; returns (new_x, BassKernelResults)."""
    nc = _get_nc()
    in_maps = make_in_maps(x, adj, W_node, W_edge, b_edge)
    res = run_bass_kernel_spmd(nc, in_maps, list(range(8)), **run_kwargs)
    new_x = np.empty((B, N, F_OUT), np.float32)
    for c in range(8):
        b, ihalf = c // 2, c % 2
        new_x[b, ihalf * IH : (ihalf + 1) * IH] = res.results[c]["out"]
    return new_x, res


def kernel(x, adj, W_node, W_edge, b_edge):
    new_x, _ = run(x, adj, W_node, W_edge, b_edge)
    return new_x, np.asarray(adj)
